# revision 18
# baseline (speedup 1.0000x reference)
"""Causal GQA attention on 8 TRN2 NeuronCores (head-sharded, no collectives).

Problem: NUM_TOKENS=2048, NUM_HEADS=32, HEAD_DIM=128, NUM_KV_HEADS=8, causal.
Sharding: core i holds KV head i and its 4 query heads (GQA group stays
together). Each core runs an independent flash-attention-style kernel:
  ST[k,q] = K @ Q^T  (bf16 matmuls, PSUM f32)
  PT      = exp(SCALE * ST)           (ACT, no max-subtraction: scores ~N(0,1))
  O[q, d+1] += PT_block^T @ [V | 1]   (ones column yields softmax denominators)
  out     = O[:, :d] / O[:, d]
The emission is software-pipelined: score matmuls of group i+1 are emitted
before the PV matmuls of group i, so the PE computes scores while ACT
exponentiates and never idles waiting for the activation.
"""

import numpy as np

import concourse.bass as bass
import concourse.bacc as bacc
import concourse.mybir as mybir
from concourse.tile import TileContext
from concourse.masks import make_identity
from concourse.bass_utils import run_bass_kernel_spmd

T = 2048          # tokens
D = 128           # head dim
HEADS = 4         # query heads per core
N_CORES = 8
W = 512           # q-chunk width
NKT = T // 128    # 16 k-tiles
NC_CHUNK = T // W
SCALE = D ** -0.5
F32 = mybir.dt.float32
BF16 = mybir.dt.bfloat16
EXP = mybir.ActivationFunctionType.Exp

# band packing: k-tile band offset r -> (packed col offset, width)
BAND_SLOTS = {0: (0, 512), 1: (512, 384), 3: (896, 128), 2: (1024, 256)}
BAND_W = 1280


def build_attention_nc():
    nc = bacc.Bacc("TRN2", target_bir_lowering=False, debug=False)

    q_in = nc.declare_dram_parameter("query", [T, HEADS * D], F32, isOutput=False)
    k_in = nc.declare_dram_parameter("key", [T, D], F32, isOutput=False)
    v_in = nc.declare_dram_parameter("value", [T, D], F32, isOutput=False)
    out = nc.declare_dram_parameter("out", [T, HEADS * D], F32, isOutput=True)

    # persistent SBUF tensors
    qt_sb = nc.alloc_sbuf_tensor("qt_sb", [128, HEADS, T], BF16).ap()   # [d, h, q]
    kt_sb = nc.alloc_sbuf_tensor("kt_sb", [128, NKT, 128], BF16).ap()   # [d, kt, k]
    v_aug = nc.alloc_sbuf_tensor("v_aug", [128, NKT, 132], BF16).ap()   # [k, kt, d+1]
    ident = nc.alloc_sbuf_tensor("ident", [128, 128], BF16).ap()

    with TileContext(nc) as tc:
        make_identity(nc, ident)
        nc.gpsimd.memset(v_aug[:, :, 128:129], 1.0)

        with (
            tc.tile_pool(name="ld", bufs=2) as ld,
            tc.tile_pool(name="ldb", bufs=2) as ldb,
            tc.tile_pool(name="st", bufs=2, space="PSUM") as stp,
            tc.tile_pool(name="ops", bufs=1, space="PSUM") as ops,
            tc.tile_pool(name="pt", bufs=3) as ptp,
            tc.tile_pool(name="osb", bufs=2) as osb,
            tc.tile_pool(name="rp", bufs=4) as rp,
        ):
            loaded = {}

            def load_kv(g):
                """Dispatch K/V DMAs for tile-group g; V copied on gpsimd."""
                rows = slice(g * 512, (g + 1) * 512)
                kf = ld.tile([128, 4, 128], F32, tag="kf", bufs=4, name="kf")
                nc.sync.dma_start(
                    out=kf[:],
                    in_=k_in[rows, :].rearrange("(t p) d -> p t d", p=128),
                )
                loaded["k%d" % g] = kf
                vf = ld.tile([128, 4, 128], F32, tag="vf", bufs=4, name="vf")
                nc.sync.dma_start(
                    out=vf[:],
                    in_=v_in[rows, :].rearrange("(t p) d -> p t d", p=128),
                )
                nc.gpsimd.tensor_copy(v_aug[:, 4 * g : 4 * g + 4, 0:128], vf[:])

            def load_q_dma(g, per_head=False):
                rows = slice(g * 512, (g + 1) * 512)
                if per_head:
                    qfs = []
                    for h in range(HEADS):
                        qfh = ld.tile([128, 4, 128], F32, tag="qf0", bufs=4, name="qfh")
                        nc.sync.dma_start(
                            out=qfh[:],
                            in_=q_in[rows, h * 128 : (h + 1) * 128].rearrange(
                                "(t p) d -> p t d", p=128
                            ),
                        )
                        qfs.append(qfh)
                    loaded[g] = lambda h: qfs[h][:, :, :]
                else:
                    qf = ld.tile([128, 4, 512], F32, tag="qf", name="qf")
                    nc.sync.dma_start(
                        out=qf[:],
                        in_=q_in[rows, :].rearrange("(t p) d -> p t d", p=128),
                    )
                    loaded[g] = lambda h: qf[:, :, h * 128 : (h + 1) * 128]

            def k_tr(g):
                kf = loaded.pop("k%d" % g)
                kb = ldb.tile([128, 4, 128], BF16, tag="kb", name="kb")
                nc.vector.tensor_copy(kb[:], kf[:])
                tr = stp.tile([128, 4, 128], BF16, tag="st", name="tr")
                for j2 in range(4):
                    nc.tensor.transpose(tr[:, j2, :], kb[:, j2, :], ident[:])
                nc.vector.tensor_copy(kt_sb[:, 4 * g : 4 * g + 4, :], tr[:])

            def q_tr(g, heads=range(HEADS)):
                """Cast to bf16 and PE-transpose Q group g into qt_sb."""
                q_slice = loaded[g]
                for h in heads:
                    qb = ldb.tile([128, 4, 128], BF16, tag="qb", name="qb")
                    nc.vector.tensor_copy(qb[:], q_slice(h))
                    tr = stp.tile([128, 4, 128], BF16, tag="st", name="tr")
                    for t in range(4):
                        nc.tensor.transpose(tr[:, t, :], qb[:, t, :], ident[:])
                    nc.vector.tensor_copy(
                        qt_sb[:, h, g * 512 : (g + 1) * 512], tr[:]
                    )

            # ---- software-pipelined attention ----
            pending = [None]

            def flush():
                if pending[0] is not None:
                    pending[0]()
                    pending[0] = None

            def attn_head(h, c):
                q0 = c * W
                state = {}
                n_grp_total = (4 * c + 2) // 3 + 1  # full groups + band
                grp_idx = [0]

                def ensure_o():
                    if "o" in state:
                        return
                    state["o"] = [
                        ops.tile([128, 2, 130], F32, tag="o01", name="o01"),
                        ops.tile([128, 2, 130], F32, tag="o23", name="o23"),
                    ]
                    nc.vector.memset(state["o"][0][:], 0.0)
                    nc.vector.memset(state["o"][1][:], 0.0)

                def pv(j, t, lhsT):
                    nc.tensor.matmul(
                        state["o"][t // 2][:, t % 2, 0:129],
                        lhsT=lhsT,
                        rhs=v_aug[:, j, 0:129],
                        start=False,
                        stop=(j == 4 * c + t),
                        skip_group_check=True,
                    )

                def finish():
                    o_ts = state["o"]
                    ot = osb.tile([128, 4, 128], F32, tag="ot", name="ot")
                    for i in range(2):
                        r = rp.tile([128, 2, 1], F32, tag="r", name="r")
                        nc.vector.reciprocal(r[:], o_ts[i][:, :, 128:129])
                        nc.vector.tensor_mul(
                            ot[:, 2 * i : 2 * i + 2, :],
                            o_ts[i][:, :, 0:128],
                            r[:].broadcast_to([128, 2, 128]),
                        )
                    nc.sync.dma_start(
                        out=out[q0 : q0 + W, h * 128 : (h + 1) * 128].rearrange(
                            "(t p) d -> p t d", p=128
                        ),
                        in_=ot[:],
                    )

                # full k-tiles, groups of 3
                for g0 in range(0, 4 * c, 3):
                    grp = list(range(g0, min(g0 + 3, 4 * c)))
                    n = len(grp)
                    st = stp.tile([128, 3, 512], F32, tag="st", name="st")
                    pt = ptp.tile([128, 3, 512], BF16, tag="pt", name="pt")
                    for j2, j in enumerate(grp):
                        nc.tensor.matmul(
                            st[:, j2, :],
                            lhsT=kt_sb[:, j, :],
                            rhs=qt_sb[:, h, q0 : q0 + W],
                            start=True,
                            stop=True,
                        )
                    nc.scalar.activation(pt[:, 0:n, :], st[:, 0:n, :], EXP, scale=SCALE)
                    flush()

                    def emit_full(grp=grp, pt=pt):
                        ensure_o()
                        for j2, j in enumerate(grp):
                            for t in range(4):
                                pv(j, t, pt[:, j2, 128 * t : 128 * (t + 1)])

                    pending[0] = emit_full

                # band k-tiles (4c..4c+3), packed exact-width layout
                stb = stp.tile([128, BAND_W], F32, tag="st", name="stb")
                ptb = ptp.tile([128, BAND_W], BF16, tag="pt", name="ptb")
                for r in range(4):
                    j = 4 * c + r
                    off, wd = BAND_SLOTS[r]
                    nc.tensor.matmul(
                        stb[:, off : off + wd],
                        lhsT=kt_sb[:, j, :],
                        rhs=qt_sb[:, h, q0 + 128 * r : q0 + W],
                        start=True,
                        stop=True,
                    )
                nc.scalar.activation(ptb[:], stb[:], EXP, scale=SCALE)
                flush()

                def emit_band(ptb=ptb):
                    ensure_o()
                    for r in range(4):
                        j = 4 * c + r
                        off, wd = BAND_SLOTS[r]
                        # diagonal 128x128 block: zero where q < k
                        blk = ptb[:, off : off + 128]
                        nc.gpsimd.affine_select(
                            out=blk,
                            in_=blk,
                            compare_op=mybir.AluOpType.is_ge,
                            fill=0.0,
                            base=0,
                            pattern=[[1, 128]],
                            channel_multiplier=-1,
                        )
                        for t in range(r, 4):
                            boff = off + 128 * (t - r)
                            pv(j, t, ptb[:, boff : boff + 128])
                    finish()

                pending[0] = emit_band

            order = [3, 2, 1, 0]
            for g in range(NC_CHUNK):
                load_kv(g)
            load_q_dma(order[0], per_head=True)
            for g in range(NC_CHUNK):
                k_tr(g)
            for ci, c in enumerate(order):
                for h in range(HEADS):
                    if ci == 0:
                        q_tr(c, heads=[h])
                    if h == 0 and ci + 1 < len(order):
                        load_q_dma(order[ci + 1])
                    if h == 2 and ci + 1 < len(order):
                        q_tr(order[ci + 1])
                    attn_head(h, c)
            flush()

    nc.compile()
    return nc


_NC_CACHE = {}


def _get_nc():
    if "nc" not in _NC_CACHE:
        _NC_CACHE["nc"] = build_attention_nc()
    return _NC_CACHE["nc"]


def shard_inputs(query, key, value):
    in_maps = []
    for i in range(N_CORES):
        in_maps.append(
            {
                "query": np.ascontiguousarray(
                    query[:, i * HEADS * D : (i + 1) * HEADS * D], dtype=np.float32
                ),
                "key": np.ascontiguousarray(key[:, i * D : (i + 1) * D], dtype=np.float32),
                "value": np.ascontiguousarray(
                    value[:, i * D : (i + 1) * D], dtype=np.float32
                ),
            }
        )
    return in_maps


def kernel(query, key, value, _trace=False):
    nc = _get_nc()
    in_maps = shard_inputs(np.asarray(query), np.asarray(key), np.asarray(value))
    res = run_bass_kernel_spmd(nc, in_maps, core_ids=list(range(N_CORES)), trace=_trace)
    full = np.concatenate([res.results[i]["out"] for i in range(N_CORES)], axis=1)
    if _trace:
        kernel.last_result = res
    return full.astype(np.float32)


# revision 20
# speedup vs baseline: 1.0470x; 1.0470x over previous
"""Causal GQA attention on 8 TRN2 NeuronCores (head-sharded, no collectives).

Problem: NUM_TOKENS=2048, NUM_HEADS=32, HEAD_DIM=128, NUM_KV_HEADS=8, causal.
Sharding: core i holds KV head i and its 4 query heads (GQA group stays
together). Each core runs an independent flash-attention-style kernel:
  ST[k,q] = K @ Q^T  (bf16 matmuls, PSUM f32)
  PT      = exp(SCALE * ST)           (ACT, no max-subtraction: scores ~N(0,1))
  O[q, d+1] += PT_block^T @ [V | 1]   (ones column yields softmax denominators)
  out     = O[:, :d] / O[:, d]
The emission is software-pipelined: score matmuls of group i+1 are emitted
before the PV matmuls of group i, so the PE computes scores while ACT
exponentiates and never idles waiting for the activation.
"""

import numpy as np

import concourse.bass as bass
import concourse.bacc as bacc
import concourse.mybir as mybir
from concourse.tile import TileContext
from concourse.masks import make_identity
from concourse.bass_utils import run_bass_kernel_spmd

T = 2048          # tokens
D = 128           # head dim
HEADS = 4         # query heads per core
N_CORES = 8
W = 512           # q-chunk width
NKT = T // 128    # 16 k-tiles
NC_CHUNK = T // W
SCALE = D ** -0.5
F32 = mybir.dt.float32
BF16 = mybir.dt.bfloat16
EXP = mybir.ActivationFunctionType.Exp

# band packing: k-tile band offset r -> (packed col offset, width)
BAND_SLOTS = {0: (0, 512), 1: (512, 384), 3: (896, 128), 2: (1024, 256)}
BAND_W = 1280


def build_attention_nc():
    nc = bacc.Bacc("TRN2", target_bir_lowering=False, debug=False)

    q_in = nc.declare_dram_parameter("query", [T, HEADS * D], F32, isOutput=False)
    k_in = nc.declare_dram_parameter("key", [T, D], F32, isOutput=False)
    v_in = nc.declare_dram_parameter("value", [T, D], F32, isOutput=False)
    out = nc.declare_dram_parameter("out", [T, HEADS * D], F32, isOutput=True)

    # persistent SBUF tensors
    qt_sb = nc.alloc_sbuf_tensor("qt_sb", [128, HEADS, T], BF16).ap()   # [d, h, q]
    kt_sb = nc.alloc_sbuf_tensor("kt_sb", [128, NKT, 128], BF16).ap()   # [d, kt, k]
    v_aug = nc.alloc_sbuf_tensor("v_aug", [128, NKT, 132], BF16).ap()   # [k, kt, d+1]
    ident = nc.alloc_sbuf_tensor("ident", [128, 128], BF16).ap()

    with TileContext(nc) as tc:
        make_identity(nc, ident)
        nc.gpsimd.memset(v_aug[:, :, 128:129], 1.0)

        with (
            tc.tile_pool(name="ld", bufs=2) as ld,
            tc.tile_pool(name="ldb", bufs=2) as ldb,
            tc.tile_pool(name="st", bufs=2, space="PSUM") as stp,
            tc.tile_pool(name="ops", bufs=1, space="PSUM") as ops,
            tc.tile_pool(name="pt", bufs=3) as ptp,
            tc.tile_pool(name="osb", bufs=2) as osb,
            tc.tile_pool(name="rp", bufs=4) as rp,
        ):
            loaded = {}

            def load_k_dma(g):
                rows = slice(g * 512, (g + 1) * 512)
                kf = ld.tile([128, 4, 128], F32, tag="kf", bufs=4, name="kf")
                nc.sync.dma_start(
                    out=kf[:],
                    in_=k_in[rows, :].rearrange("(t p) d -> p t d", p=128),
                )
                loaded["k%d" % g] = kf

            def load_v_dma(g):
                rows = slice(g * 512, (g + 1) * 512)
                vf = ld.tile([128, 4, 128], F32, tag="vf", bufs=4, name="vf")
                nc.sync.dma_start(
                    out=vf[:],
                    in_=v_in[rows, :].rearrange("(t p) d -> p t d", p=128),
                )
                loaded["v%d" % g] = vf

            def v_copy(g):
                vf = loaded.pop("v%d" % g)
                nc.vector.tensor_copy(v_aug[:, 4 * g : 4 * g + 4, 0:128], vf[:])

            def load_q_dma(g, per_head=False, interleave_k=False):
                rows = slice(g * 512, (g + 1) * 512)
                if per_head:
                    qfs = []
                    for h in range(HEADS):
                        if interleave_k:
                            load_k_dma(h)
                        qfh = ld.tile([128, 4, 128], F32, tag="qf0", bufs=4, name="qfh")
                        nc.sync.dma_start(
                            out=qfh[:],
                            in_=q_in[rows, h * 128 : (h + 1) * 128].rearrange(
                                "(t p) d -> p t d", p=128
                            ),
                        )
                        qfs.append(qfh)
                    loaded[g] = lambda h: qfs[h][:, :, :]
                else:
                    qf = ld.tile([128, 4, 512], F32, tag="qf", name="qf")
                    nc.sync.dma_start(
                        out=qf[:],
                        in_=q_in[rows, :].rearrange("(t p) d -> p t d", p=128),
                    )
                    loaded[g] = lambda h: qf[:, :, h * 128 : (h + 1) * 128]

            def k_tr(g):
                kf = loaded.pop("k%d" % g)
                kb = ldb.tile([128, 4, 128], BF16, tag="kb", name="kb")
                nc.vector.tensor_copy(kb[:], kf[:])
                tr = stp.tile([128, 4, 128], BF16, tag="st", name="tr")
                for j2 in range(4):
                    nc.tensor.transpose(tr[:, j2, :], kb[:, j2, :], ident[:])
                nc.vector.tensor_copy(kt_sb[:, 4 * g : 4 * g + 4, :], tr[:])

            def q_tr(g, heads=range(HEADS)):
                """Cast to bf16 and PE-transpose Q group g into qt_sb."""
                q_slice = loaded[g]
                for h in heads:
                    qb = ldb.tile([128, 4, 128], BF16, tag="qb", name="qb")
                    nc.vector.tensor_copy(qb[:], q_slice(h))
                    tr = stp.tile([128, 4, 128], BF16, tag="st", name="tr")
                    for t in range(4):
                        nc.tensor.transpose(tr[:, t, :], qb[:, t, :], ident[:])
                    nc.vector.tensor_copy(
                        qt_sb[:, h, g * 512 : (g + 1) * 512], tr[:]
                    )

            # ---- software-pipelined attention ----
            pending = [None]

            def flush():
                if pending[0] is not None:
                    pending[0]()
                    pending[0] = None

            def attn_head(h, c):
                q0 = c * W
                state = {}
                n_grp_total = (4 * c + 2) // 3 + 1  # full groups + band
                grp_idx = [0]

                def ensure_o():
                    if "o" in state:
                        return
                    state["o"] = [
                        ops.tile([128, 2, 130], F32, tag="o01", name="o01"),
                        ops.tile([128, 2, 130], F32, tag="o23", name="o23"),
                    ]
                    nc.vector.memset(state["o"][0][:], 0.0)
                    nc.vector.memset(state["o"][1][:], 0.0)

                def pv(j, t, lhsT):
                    nc.tensor.matmul(
                        state["o"][t // 2][:, t % 2, 0:129],
                        lhsT=lhsT,
                        rhs=v_aug[:, j, 0:129],
                        start=False,
                        stop=(j == 4 * c + t),
                        skip_group_check=True,
                    )

                def finish():
                    o_ts = state["o"]
                    ot = osb.tile([128, 4, 128], F32, tag="ot", name="ot")
                    for i in range(2):
                        r = rp.tile([128, 2, 1], F32, tag="r", name="r")
                        nc.vector.reciprocal(r[:], o_ts[i][:, :, 128:129])
                        nc.vector.tensor_mul(
                            ot[:, 2 * i : 2 * i + 2, :],
                            o_ts[i][:, :, 0:128],
                            r[:].broadcast_to([128, 2, 128]),
                        )
                    nc.sync.dma_start(
                        out=out[q0 : q0 + W, h * 128 : (h + 1) * 128].rearrange(
                            "(t p) d -> p t d", p=128
                        ),
                        in_=ot[:],
                    )

                # full k-tiles, groups of 3
                for g0 in range(0, 4 * c, 3):
                    grp = list(range(g0, min(g0 + 3, 4 * c)))
                    n = len(grp)
                    st = stp.tile([128, 3, 512], F32, tag="st", name="st")
                    pt = ptp.tile([128, 3, 512], BF16, tag="pt", name="pt")
                    for j2, j in enumerate(grp):
                        nc.tensor.matmul(
                            st[:, j2, :],
                            lhsT=kt_sb[:, j, :],
                            rhs=qt_sb[:, h, q0 : q0 + W],
                            start=True,
                            stop=True,
                        )
                    nc.scalar.activation(pt[:, 0:n, :], st[:, 0:n, :], EXP, scale=SCALE)
                    flush()

                    def emit_full(grp=grp, pt=pt):
                        ensure_o()
                        for j2, j in enumerate(grp):
                            for t in range(4):
                                pv(j, t, pt[:, j2, 128 * t : 128 * (t + 1)])

                    pending[0] = emit_full

                # band k-tiles (4c..4c+3), packed exact-width layout
                stb = stp.tile([128, BAND_W], F32, tag="st", name="stb")
                ptb = ptp.tile([128, BAND_W], BF16, tag="pt", name="ptb")
                for r in range(4):
                    j = 4 * c + r
                    off, wd = BAND_SLOTS[r]
                    nc.tensor.matmul(
                        stb[:, off : off + wd],
                        lhsT=kt_sb[:, j, :],
                        rhs=qt_sb[:, h, q0 + 128 * r : q0 + W],
                        start=True,
                        stop=True,
                    )
                nc.scalar.activation(ptb[:], stb[:], EXP, scale=SCALE)
                flush()

                def emit_band(ptb=ptb):
                    ensure_o()
                    for r in range(4):
                        j = 4 * c + r
                        off, wd = BAND_SLOTS[r]
                        # diagonal 128x128 block: zero where q < k
                        blk = ptb[:, off : off + 128]
                        nc.gpsimd.affine_select(
                            out=blk,
                            in_=blk,
                            compare_op=mybir.AluOpType.is_ge,
                            fill=0.0,
                            base=0,
                            pattern=[[1, 128]],
                            channel_multiplier=-1,
                        )
                        for t in range(r, 4):
                            boff = off + 128 * (t - r)
                            pv(j, t, ptb[:, boff : boff + 128])
                    finish()

                pending[0] = emit_band

            order = [3, 2, 1, 0]
            # interleaved dispatch: K groups and first-chunk per-head Q
            load_q_dma(order[0], per_head=True, interleave_k=True)
            for g in range(NC_CHUNK):
                load_v_dma(g)
            for g in range(NC_CHUNK):
                k_tr(g)
            for ci, c in enumerate(order):
                for h in range(HEADS):
                    if ci == 0:
                        q_tr(c, heads=[h])
                        if h == 0:
                            v_copy(0), v_copy(1), v_copy(2)
                        elif h == 1:
                            v_copy(3)
                    if h == 0 and ci + 1 < len(order):
                        load_q_dma(order[ci + 1])
                    if h == 1 and ci + 1 < len(order):
                        q_tr(order[ci + 1], heads=[0, 1])
                    if h == 2 and ci + 1 < len(order):
                        q_tr(order[ci + 1], heads=[2, 3])
                    attn_head(h, c)
            flush()

    nc.compile()
    return nc


_NC_CACHE = {}


def _get_nc():
    if "nc" not in _NC_CACHE:
        _NC_CACHE["nc"] = build_attention_nc()
    return _NC_CACHE["nc"]


def shard_inputs(query, key, value):
    in_maps = []
    for i in range(N_CORES):
        in_maps.append(
            {
                "query": np.ascontiguousarray(
                    query[:, i * HEADS * D : (i + 1) * HEADS * D], dtype=np.float32
                ),
                "key": np.ascontiguousarray(key[:, i * D : (i + 1) * D], dtype=np.float32),
                "value": np.ascontiguousarray(
                    value[:, i * D : (i + 1) * D], dtype=np.float32
                ),
            }
        )
    return in_maps


def kernel(query, key, value, _trace=False):
    nc = _get_nc()
    in_maps = shard_inputs(np.asarray(query), np.asarray(key), np.asarray(value))
    res = run_bass_kernel_spmd(nc, in_maps, core_ids=list(range(N_CORES)), trace=_trace)
    full = np.concatenate([res.results[i]["out"] for i in range(N_CORES)], axis=1)
    if _trace:
        kernel.last_result = res
    return full.astype(np.float32)


# revision 21
# speedup vs baseline: 1.0539x; 1.0065x over previous
"""Causal GQA attention on 8 TRN2 NeuronCores (head-sharded, no collectives).

Problem: NUM_TOKENS=2048, NUM_HEADS=32, HEAD_DIM=128, NUM_KV_HEADS=8, causal.
Sharding: core i holds KV head i and its 4 query heads (GQA group stays
together). Each core runs an independent flash-attention-style kernel:
  ST[k,q] = K @ Q^T  (bf16 matmuls, PSUM f32)
  PT      = exp(SCALE * ST)           (ACT, no max-subtraction: scores ~N(0,1))
  O[q, d+1] += PT_block^T @ [V | 1]   (ones column yields softmax denominators)
  out     = O[:, :d] / O[:, d]
The emission is software-pipelined: score matmuls of group i+1 are emitted
before the PV matmuls of group i, so the PE computes scores while ACT
exponentiates and never idles waiting for the activation.
"""

import numpy as np

import concourse.bass as bass
import concourse.bacc as bacc
import concourse.mybir as mybir
from concourse.tile import TileContext
from concourse.masks import make_identity
from concourse.bass_utils import run_bass_kernel_spmd

T = 2048          # tokens
D = 128           # head dim
HEADS = 4         # query heads per core
N_CORES = 8
W = 512           # q-chunk width
NKT = T // 128    # 16 k-tiles
NC_CHUNK = T // W
SCALE = D ** -0.5
F32 = mybir.dt.float32
BF16 = mybir.dt.bfloat16
EXP = mybir.ActivationFunctionType.Exp

# band packing: k-tile band offset r -> (packed col offset, width)
BAND_SLOTS = {0: (0, 512), 1: (512, 384), 3: (896, 128), 2: (1024, 256)}
BAND_W = 1280


def build_attention_nc():
    nc = bacc.Bacc("TRN2", target_bir_lowering=False, debug=False)

    q_in = nc.declare_dram_parameter("query", [T, HEADS * D], F32, isOutput=False)
    k_in = nc.declare_dram_parameter("key", [T, D], F32, isOutput=False)
    v_in = nc.declare_dram_parameter("value", [T, D], F32, isOutput=False)
    out = nc.declare_dram_parameter("out", [T, HEADS * D], F32, isOutput=True)

    # persistent SBUF tensors
    qt_sb = nc.alloc_sbuf_tensor("qt_sb", [128, HEADS, T], BF16).ap()   # [d, h, q]
    kt_sb = nc.alloc_sbuf_tensor("kt_sb", [128, NKT, 128], BF16).ap()   # [d, kt, k]
    v_aug = nc.alloc_sbuf_tensor("v_aug", [128, NKT, 132], BF16).ap()   # [k, kt, d+1]
    ident = nc.alloc_sbuf_tensor("ident", [128, 128], BF16).ap()

    with TileContext(nc) as tc:
        make_identity(nc, ident)
        nc.gpsimd.memset(v_aug[:, :, 128:129], 1.0)

        with (
            tc.tile_pool(name="ld", bufs=2) as ld,
            tc.tile_pool(name="ldb", bufs=2) as ldb,
            tc.tile_pool(name="st", bufs=2, space="PSUM") as stp,
            tc.tile_pool(name="ops", bufs=1, space="PSUM") as ops,
            tc.tile_pool(name="pt", bufs=3) as ptp,
            tc.tile_pool(name="osb", bufs=2) as osb,
            tc.tile_pool(name="rp", bufs=4) as rp,
        ):
            loaded = {}

            def load_k_dma(g):
                rows = slice(g * 512, (g + 1) * 512)
                kf = ld.tile([128, 4, 128], F32, tag="kf", bufs=4, name="kf")
                nc.sync.dma_start(
                    out=kf[:],
                    in_=k_in[rows, :].rearrange("(t p) d -> p t d", p=128),
                )
                loaded["k%d" % g] = kf

            def load_v_dma(g):
                rows = slice(g * 512, (g + 1) * 512)
                vf = ld.tile([128, 4, 128], F32, tag="vf", bufs=4, name="vf")
                nc.sync.dma_start(
                    out=vf[:],
                    in_=v_in[rows, :].rearrange("(t p) d -> p t d", p=128),
                )
                loaded["v%d" % g] = vf

            def v_copy(g):
                vf = loaded.pop("v%d" % g)
                nc.vector.tensor_copy(v_aug[:, 4 * g : 4 * g + 4, 0:128], vf[:])

            def load_q_dma(g, per_head=False, interleave_k=False):
                rows = slice(g * 512, (g + 1) * 512)
                if per_head:
                    qfs = []
                    for h in range(HEADS):
                        if interleave_k:
                            load_k_dma(h)
                        qfh = ld.tile([128, 4, 128], F32, tag="qf0", bufs=4, name="qfh")
                        nc.sync.dma_start(
                            out=qfh[:],
                            in_=q_in[rows, h * 128 : (h + 1) * 128].rearrange(
                                "(t p) d -> p t d", p=128
                            ),
                        )
                        qfs.append(qfh)
                    loaded[g] = lambda h: qfs[h][:, :, :]
                else:
                    qf = ld.tile([128, 4, 512], F32, tag="qf", name="qf")
                    nc.sync.dma_start(
                        out=qf[:],
                        in_=q_in[rows, :].rearrange("(t p) d -> p t d", p=128),
                    )
                    loaded[g] = lambda h: qf[:, :, h * 128 : (h + 1) * 128]

            def k_tr(g):
                kf = loaded.pop("k%d" % g)
                kb = ldb.tile([128, 4, 128], BF16, tag="kb", name="kb")
                nc.vector.tensor_copy(kb[:], kf[:])
                tr = stp.tile([128, 4, 128], BF16, tag="st", name="tr")
                for j2 in range(4):
                    nc.tensor.transpose(tr[:, j2, :], kb[:, j2, :], ident[:])
                nc.vector.tensor_copy(kt_sb[:, 4 * g : 4 * g + 4, :], tr[:])

            def q_tr(g, heads=range(HEADS)):
                """Cast to bf16 and PE-transpose Q group g into qt_sb."""
                q_slice = loaded[g]
                for h in heads:
                    qb = ldb.tile([128, 4, 128], BF16, tag="qb", name="qb")
                    nc.vector.tensor_copy(qb[:], q_slice(h))
                    tr = stp.tile([128, 4, 128], BF16, tag="st", name="tr")
                    for t in range(4):
                        nc.tensor.transpose(tr[:, t, :], qb[:, t, :], ident[:])
                    nc.vector.tensor_copy(
                        qt_sb[:, h, g * 512 : (g + 1) * 512], tr[:]
                    )

            # ---- software-pipelined attention ----
            pending = [None]

            def flush():
                if pending[0] is not None:
                    pending[0]()
                    pending[0] = None

            def attn_head(h, c, inject=None):
                q0 = c * W
                state = {}
                n_grp_total = (4 * c + 2) // 3 + 1  # full groups + band
                grp_idx = [0]

                def ensure_o():
                    if "o" in state:
                        return
                    state["o"] = [
                        ops.tile([128, 2, 130], F32, tag="o01", name="o01"),
                        ops.tile([128, 2, 130], F32, tag="o23", name="o23"),
                    ]
                    nc.vector.memset(state["o"][0][:], 0.0)
                    nc.vector.memset(state["o"][1][:], 0.0)

                def pv(j, t, lhsT):
                    nc.tensor.matmul(
                        state["o"][t // 2][:, t % 2, 0:129],
                        lhsT=lhsT,
                        rhs=v_aug[:, j, 0:129],
                        start=False,
                        stop=(j == 4 * c + t),
                        skip_group_check=True,
                    )

                def finish():
                    o_ts = state["o"]
                    ot = osb.tile([128, 4, 128], F32, tag="ot", name="ot")
                    for i in range(2):
                        r = rp.tile([128, 2, 1], F32, tag="r", name="r")
                        nc.vector.reciprocal(r[:], o_ts[i][:, :, 128:129])
                        nc.vector.tensor_mul(
                            ot[:, 2 * i : 2 * i + 2, :],
                            o_ts[i][:, :, 0:128],
                            r[:].broadcast_to([128, 2, 128]),
                        )
                    nc.sync.dma_start(
                        out=out[q0 : q0 + W, h * 128 : (h + 1) * 128].rearrange(
                            "(t p) d -> p t d", p=128
                        ),
                        in_=ot[:],
                    )

                # full k-tiles, groups of 3
                for gi, g0 in enumerate(range(0, 4 * c, 3)):
                    if inject and gi in inject:
                        inject[gi]()
                    grp = list(range(g0, min(g0 + 3, 4 * c)))
                    n = len(grp)
                    st = stp.tile([128, 3, 512], F32, tag="st", name="st")
                    pt = ptp.tile([128, 3, 512], BF16, tag="pt", name="pt")
                    for j2, j in enumerate(grp):
                        nc.tensor.matmul(
                            st[:, j2, :],
                            lhsT=kt_sb[:, j, :],
                            rhs=qt_sb[:, h, q0 : q0 + W],
                            start=True,
                            stop=True,
                        )
                    nc.scalar.activation(pt[:, 0:n, :], st[:, 0:n, :], EXP, scale=SCALE)
                    flush()

                    def emit_full(grp=grp, pt=pt):
                        ensure_o()
                        for j2, j in enumerate(grp):
                            for t in range(4):
                                pv(j, t, pt[:, j2, 128 * t : 128 * (t + 1)])

                    pending[0] = emit_full

                # band k-tiles (4c..4c+3), packed exact-width layout
                stb = stp.tile([128, BAND_W], F32, tag="st", name="stb")
                ptb = ptp.tile([128, BAND_W], BF16, tag="pt", name="ptb")
                for r in range(4):
                    j = 4 * c + r
                    off, wd = BAND_SLOTS[r]
                    nc.tensor.matmul(
                        stb[:, off : off + wd],
                        lhsT=kt_sb[:, j, :],
                        rhs=qt_sb[:, h, q0 + 128 * r : q0 + W],
                        start=True,
                        stop=True,
                    )
                nc.scalar.activation(ptb[:], stb[:], EXP, scale=SCALE)
                flush()

                def emit_band(ptb=ptb):
                    ensure_o()
                    for r in range(4):
                        # diagonal 128x128 block: zero where q < k
                        off, _ = BAND_SLOTS[r]
                        blk = ptb[:, off : off + 128]
                        nc.gpsimd.affine_select(
                            out=blk,
                            in_=blk,
                            compare_op=mybir.AluOpType.is_ge,
                            fill=0.0,
                            base=0,
                            pattern=[[1, 128]],
                            channel_multiplier=-1,
                        )
                    for r in range(4):  # off-diagonal blocks: not mask-gated
                        j = 4 * c + r
                        off, _ = BAND_SLOTS[r]
                        for t in range(r + 1, 4):
                            boff = off + 128 * (t - r)
                            pv(j, t, ptb[:, boff : boff + 128])
                    for r in range(4):  # diagonal blocks last
                        j = 4 * c + r
                        off, _ = BAND_SLOTS[r]
                        pv(j, r, ptb[:, off : off + 128])
                    finish()

                pending[0] = emit_band

            order = [3, 2, 1, 0]
            # interleaved dispatch: K groups and first-chunk per-head Q
            load_q_dma(order[0], per_head=True, interleave_k=True)
            for g in range(NC_CHUNK):
                load_v_dma(g)
            k_tr(0)
            for ci, c in enumerate(order):
                for h in range(HEADS):
                    inject = None
                    if ci == 0:
                        q_tr(c, heads=[h])
                        if h == 0:
                            v_copy(0), v_copy(1), v_copy(2)
                            inject = {1: lambda: k_tr(1), 2: lambda: k_tr(2), 3: lambda: k_tr(3)}
                        elif h == 1:
                            v_copy(3)
                    if h == 0 and ci + 1 < len(order):
                        load_q_dma(order[ci + 1])
                    if h == 1 and ci + 1 < len(order):
                        q_tr(order[ci + 1], heads=[0, 1])
                    if h == 2 and ci + 1 < len(order):
                        q_tr(order[ci + 1], heads=[2, 3])
                    attn_head(h, c, inject=inject)
            flush()

    nc.compile()
    return nc


_NC_CACHE = {}


def _get_nc():
    if "nc" not in _NC_CACHE:
        _NC_CACHE["nc"] = build_attention_nc()
    return _NC_CACHE["nc"]


def shard_inputs(query, key, value):
    in_maps = []
    for i in range(N_CORES):
        in_maps.append(
            {
                "query": np.ascontiguousarray(
                    query[:, i * HEADS * D : (i + 1) * HEADS * D], dtype=np.float32
                ),
                "key": np.ascontiguousarray(key[:, i * D : (i + 1) * D], dtype=np.float32),
                "value": np.ascontiguousarray(
                    value[:, i * D : (i + 1) * D], dtype=np.float32
                ),
            }
        )
    return in_maps


def kernel(query, key, value, _trace=False):
    nc = _get_nc()
    in_maps = shard_inputs(np.asarray(query), np.asarray(key), np.asarray(value))
    res = run_bass_kernel_spmd(nc, in_maps, core_ids=list(range(N_CORES)), trace=_trace)
    full = np.concatenate([res.results[i]["out"] for i in range(N_CORES)], axis=1)
    if _trace:
        kernel.last_result = res
    return full.astype(np.float32)


# revision 22
# speedup vs baseline: 1.0599x; 1.0057x over previous
"""Causal GQA attention on 8 TRN2 NeuronCores (head-sharded, no collectives).

Problem: NUM_TOKENS=2048, NUM_HEADS=32, HEAD_DIM=128, NUM_KV_HEADS=8, causal.
Sharding: core i holds KV head i and its 4 query heads (GQA group stays
together). Each core runs an independent flash-attention-style kernel:
  ST[k,q] = K @ Q^T  (bf16 matmuls, PSUM f32)
  PT      = exp(SCALE * ST)           (ACT, no max-subtraction: scores ~N(0,1))
  O[q, d+1] += PT_block^T @ [V | 1]   (ones column yields softmax denominators)
  out     = O[:, :d] / O[:, d]
The emission is software-pipelined: score matmuls of group i+1 are emitted
before the PV matmuls of group i, so the PE computes scores while ACT
exponentiates and never idles waiting for the activation.
"""

import numpy as np

import concourse.bass as bass
import concourse.bacc as bacc
import concourse.mybir as mybir
from concourse.tile import TileContext
from concourse.masks import make_identity
from concourse.bass_utils import run_bass_kernel_spmd

T = 2048          # tokens
D = 128           # head dim
HEADS = 4         # query heads per core
N_CORES = 8
W = 512           # q-chunk width
NKT = T // 128    # 16 k-tiles
NC_CHUNK = T // W
SCALE = D ** -0.5
F32 = mybir.dt.float32
BF16 = mybir.dt.bfloat16
EXP = mybir.ActivationFunctionType.Exp

# band packing: k-tile band offset r -> (packed col offset, width)
BAND_SLOTS = {0: (0, 512), 1: (512, 384), 3: (896, 128), 2: (1024, 256)}
BAND_W = 1280


def build_attention_nc():
    nc = bacc.Bacc("TRN2", target_bir_lowering=False, debug=False)

    q_in = nc.declare_dram_parameter("query", [T, HEADS * D], F32, isOutput=False)
    k_in = nc.declare_dram_parameter("key", [T, D], F32, isOutput=False)
    v_in = nc.declare_dram_parameter("value", [T, D], F32, isOutput=False)
    out = nc.declare_dram_parameter("out", [T, HEADS * D], F32, isOutput=True)

    # persistent SBUF tensors
    qt_sb = nc.alloc_sbuf_tensor("qt_sb", [128, HEADS, T], BF16).ap()   # [d, h, q]
    kt_sb = nc.alloc_sbuf_tensor("kt_sb", [128, NKT, 128], BF16).ap()   # [d, kt, k]
    v_aug = nc.alloc_sbuf_tensor("v_aug", [128, NKT, 132], BF16).ap()   # [k, kt, d+1]
    ident = nc.alloc_sbuf_tensor("ident", [128, 128], BF16).ap()

    with TileContext(nc) as tc:
        make_identity(nc, ident)
        nc.gpsimd.memset(v_aug[:, :, 128:129], 1.0)

        with (
            tc.tile_pool(name="ld", bufs=2) as ld,
            tc.tile_pool(name="ldb", bufs=3) as ldb,
            tc.tile_pool(name="st", bufs=2, space="PSUM") as stp,
            tc.tile_pool(name="ops", bufs=1, space="PSUM") as ops,
            tc.tile_pool(name="pt", bufs=4) as ptp,
            tc.tile_pool(name="osb", bufs=3) as osb,
            tc.tile_pool(name="rp", bufs=4) as rp,
        ):
            loaded = {}

            def load_k_dma(g):
                rows = slice(g * 512, (g + 1) * 512)
                kf = ld.tile([128, 4, 128], F32, tag="kf", bufs=4, name="kf")
                nc.sync.dma_start(
                    out=kf[:],
                    in_=k_in[rows, :].rearrange("(t p) d -> p t d", p=128),
                )
                loaded["k%d" % g] = kf

            def load_v_dma(g):
                rows = slice(g * 512, (g + 1) * 512)
                vf = ld.tile([128, 4, 128], F32, tag="vf", bufs=4, name="vf")
                nc.sync.dma_start(
                    out=vf[:],
                    in_=v_in[rows, :].rearrange("(t p) d -> p t d", p=128),
                )
                loaded["v%d" % g] = vf

            def v_copy(g):
                vf = loaded.pop("v%d" % g)
                nc.vector.tensor_copy(v_aug[:, 4 * g : 4 * g + 4, 0:128], vf[:])

            def load_q_dma(g, per_head=False, interleave_k=False):
                rows = slice(g * 512, (g + 1) * 512)
                if per_head:
                    qfs = []
                    for h in range(HEADS):
                        if interleave_k:
                            load_k_dma(h)
                        qfh = ld.tile([128, 4, 128], F32, tag="qf0", bufs=4, name="qfh")
                        nc.sync.dma_start(
                            out=qfh[:],
                            in_=q_in[rows, h * 128 : (h + 1) * 128].rearrange(
                                "(t p) d -> p t d", p=128
                            ),
                        )
                        qfs.append(qfh)
                    loaded[g] = lambda h: qfs[h][:, :, :]
                else:
                    qf = ld.tile([128, 4, 512], F32, tag="qf", name="qf")
                    nc.sync.dma_start(
                        out=qf[:],
                        in_=q_in[rows, :].rearrange("(t p) d -> p t d", p=128),
                    )
                    loaded[g] = lambda h: qf[:, :, h * 128 : (h + 1) * 128]

            def k_tr(g):
                kf = loaded.pop("k%d" % g)
                kb = ldb.tile([128, 4, 128], BF16, tag="kb", name="kb")
                nc.vector.tensor_copy(kb[:], kf[:])
                tr = stp.tile([128, 4, 128], BF16, tag="st", name="tr")
                for j2 in range(4):
                    nc.tensor.transpose(tr[:, j2, :], kb[:, j2, :], ident[:])
                nc.vector.tensor_copy(kt_sb[:, 4 * g : 4 * g + 4, :], tr[:])

            def q_tr(g, heads=range(HEADS)):
                """Cast to bf16 and PE-transpose Q group g into qt_sb."""
                q_slice = loaded[g]
                for h in heads:
                    qb = ldb.tile([128, 4, 128], BF16, tag="qb", name="qb")
                    nc.vector.tensor_copy(qb[:], q_slice(h))
                    tr = stp.tile([128, 4, 128], BF16, tag="st", name="tr")
                    for t in range(4):
                        nc.tensor.transpose(tr[:, t, :], qb[:, t, :], ident[:])
                    nc.vector.tensor_copy(
                        qt_sb[:, h, g * 512 : (g + 1) * 512], tr[:]
                    )

            # ---- software-pipelined attention ----
            pending = [None]

            def flush():
                if pending[0] is not None:
                    pending[0]()
                    pending[0] = None

            def attn_head(h, c, inject=None):
                q0 = c * W
                state = {}
                n_grp_total = (4 * c + 2) // 3 + 1  # full groups + band
                grp_idx = [0]

                def ensure_o():
                    if "o" in state:
                        return
                    state["o"] = [
                        ops.tile([128, 2, 130], F32, tag="o01", name="o01"),
                        ops.tile([128, 2, 130], F32, tag="o23", name="o23"),
                    ]
                    nc.vector.memset(state["o"][0][:], 0.0)
                    nc.vector.memset(state["o"][1][:], 0.0)

                def pv(j, t, lhsT):
                    nc.tensor.matmul(
                        state["o"][t // 2][:, t % 2, 0:129],
                        lhsT=lhsT,
                        rhs=v_aug[:, j, 0:129],
                        start=False,
                        stop=(j == 4 * c + t),
                        skip_group_check=True,
                    )

                def finish():
                    o_ts = state["o"]
                    ot = osb.tile([128, 4, 128], F32, tag="ot", name="ot")
                    for i in range(2):
                        r = rp.tile([128, 2, 1], F32, tag="r", name="r")
                        nc.vector.reciprocal(r[:], o_ts[i][:, :, 128:129])
                        nc.vector.tensor_mul(
                            ot[:, 2 * i : 2 * i + 2, :],
                            o_ts[i][:, :, 0:128],
                            r[:].broadcast_to([128, 2, 128]),
                        )
                    nc.sync.dma_start(
                        out=out[q0 : q0 + W, h * 128 : (h + 1) * 128].rearrange(
                            "(t p) d -> p t d", p=128
                        ),
                        in_=ot[:],
                    )

                # full k-tiles, groups of 3
                for gi, g0 in enumerate(range(0, 4 * c, 3)):
                    if inject and gi in inject:
                        inject[gi]()
                    grp = list(range(g0, min(g0 + 3, 4 * c)))
                    n = len(grp)
                    st = stp.tile([128, 3, 512], F32, tag="st", name="st")
                    pt = ptp.tile([128, 3, 512], BF16, tag="pt", name="pt")
                    for j2, j in enumerate(grp):
                        nc.tensor.matmul(
                            st[:, j2, :],
                            lhsT=kt_sb[:, j, :],
                            rhs=qt_sb[:, h, q0 : q0 + W],
                            start=True,
                            stop=True,
                        )
                    nc.scalar.activation(pt[:, 0:n, :], st[:, 0:n, :], EXP, scale=SCALE)
                    flush()

                    def emit_full(grp=grp, pt=pt):
                        ensure_o()
                        for j2, j in enumerate(grp):
                            for t in range(4):
                                pv(j, t, pt[:, j2, 128 * t : 128 * (t + 1)])

                    pending[0] = emit_full

                # band k-tiles (4c..4c+3), packed exact-width layout
                stb = stp.tile([128, BAND_W], F32, tag="st", name="stb")
                ptb = ptp.tile([128, BAND_W], BF16, tag="pt", name="ptb")
                for r in range(4):
                    j = 4 * c + r
                    off, wd = BAND_SLOTS[r]
                    nc.tensor.matmul(
                        stb[:, off : off + wd],
                        lhsT=kt_sb[:, j, :],
                        rhs=qt_sb[:, h, q0 + 128 * r : q0 + W],
                        start=True,
                        stop=True,
                    )
                nc.scalar.activation(ptb[:], stb[:], EXP, scale=SCALE)
                flush()

                def emit_band(ptb=ptb):
                    ensure_o()
                    for r in range(4):
                        # diagonal 128x128 block: zero where q < k
                        off, _ = BAND_SLOTS[r]
                        blk = ptb[:, off : off + 128]
                        nc.gpsimd.affine_select(
                            out=blk,
                            in_=blk,
                            compare_op=mybir.AluOpType.is_ge,
                            fill=0.0,
                            base=0,
                            pattern=[[1, 128]],
                            channel_multiplier=-1,
                        )
                    for r in range(4):  # off-diagonal blocks: not mask-gated
                        j = 4 * c + r
                        off, _ = BAND_SLOTS[r]
                        for t in range(r + 1, 4):
                            boff = off + 128 * (t - r)
                            pv(j, t, ptb[:, boff : boff + 128])
                    for r in range(4):  # diagonal blocks last
                        j = 4 * c + r
                        off, _ = BAND_SLOTS[r]
                        pv(j, r, ptb[:, off : off + 128])
                    finish()

                pending[0] = emit_band

            order = [3, 2, 1, 0]
            # interleaved dispatch: K groups and first-chunk per-head Q
            load_q_dma(order[0], per_head=True, interleave_k=True)
            for g in range(NC_CHUNK):
                load_v_dma(g)
            k_tr(0)
            for ci, c in enumerate(order):
                for h in range(HEADS):
                    inject = None
                    if ci == 0:
                        q_tr(c, heads=[h])
                        if h == 0:
                            v_copy(0), v_copy(1), v_copy(2)
                            inject = {1: lambda: k_tr(1), 2: lambda: k_tr(2), 3: lambda: k_tr(3)}
                        elif h == 1:
                            v_copy(3)
                    if h == 0 and ci + 1 < len(order):
                        load_q_dma(order[ci + 1])
                    if h == 1 and ci + 1 < len(order):
                        q_tr(order[ci + 1], heads=[0, 1])
                    if h == 2 and ci + 1 < len(order):
                        q_tr(order[ci + 1], heads=[2, 3])
                    attn_head(h, c, inject=inject)
            flush()

    nc.compile()
    return nc


_NC_CACHE = {}


def _get_nc():
    if "nc" not in _NC_CACHE:
        _NC_CACHE["nc"] = build_attention_nc()
    return _NC_CACHE["nc"]


def shard_inputs(query, key, value):
    in_maps = []
    for i in range(N_CORES):
        in_maps.append(
            {
                "query": np.ascontiguousarray(
                    query[:, i * HEADS * D : (i + 1) * HEADS * D], dtype=np.float32
                ),
                "key": np.ascontiguousarray(key[:, i * D : (i + 1) * D], dtype=np.float32),
                "value": np.ascontiguousarray(
                    value[:, i * D : (i + 1) * D], dtype=np.float32
                ),
            }
        )
    return in_maps


def kernel(query, key, value, _trace=False):
    nc = _get_nc()
    in_maps = shard_inputs(np.asarray(query), np.asarray(key), np.asarray(value))
    res = run_bass_kernel_spmd(nc, in_maps, core_ids=list(range(N_CORES)), trace=_trace)
    full = np.concatenate([res.results[i]["out"] for i in range(N_CORES)], axis=1)
    if _trace:
        kernel.last_result = res
    return full.astype(np.float32)


# revision 23
# speedup vs baseline: 1.0639x; 1.0038x over previous
"""Causal GQA attention on 8 TRN2 NeuronCores (head-sharded, no collectives).

Problem: NUM_TOKENS=2048, NUM_HEADS=32, HEAD_DIM=128, NUM_KV_HEADS=8, causal.
Sharding: core i holds KV head i and its 4 query heads (GQA group stays
together). Each core runs an independent flash-attention-style kernel:
  ST[k,q] = K @ Q^T  (bf16 matmuls, PSUM f32)
  PT      = exp(SCALE * ST)           (ACT, no max-subtraction: scores ~N(0,1))
  O[q, d+1] += PT_block^T @ [V | 1]   (ones column yields softmax denominators)
  out     = O[:, :d] / O[:, d]
The emission is software-pipelined: score matmuls of group i+1 are emitted
before the PV matmuls of group i, so the PE computes scores while ACT
exponentiates and never idles waiting for the activation.
"""

import numpy as np

import concourse.bass as bass
import concourse.bacc as bacc
import concourse.mybir as mybir
from concourse.tile import TileContext
from concourse.masks import make_identity
from concourse.bass_utils import run_bass_kernel_spmd

T = 2048          # tokens
D = 128           # head dim
HEADS = 4         # query heads per core
N_CORES = 8
W = 512           # q-chunk width
NKT = T // 128    # 16 k-tiles
NC_CHUNK = T // W
SCALE = D ** -0.5
F32 = mybir.dt.float32
BF16 = mybir.dt.bfloat16
EXP = mybir.ActivationFunctionType.Exp

# band packing: k-tile band offset r -> (packed col offset, width)
BAND_SLOTS = {0: (0, 512), 1: (512, 384), 3: (896, 128), 2: (1024, 256)}
BAND_W = 1280


def build_attention_nc():
    nc = bacc.Bacc("TRN2", target_bir_lowering=False, debug=False)

    q_in = nc.declare_dram_parameter("query", [T, HEADS * D], F32, isOutput=False)
    k_in = nc.declare_dram_parameter("key", [T, D], F32, isOutput=False)
    v_in = nc.declare_dram_parameter("value", [T, D], F32, isOutput=False)
    out = nc.declare_dram_parameter("out", [T, HEADS * D], F32, isOutput=True)

    # persistent SBUF tensors
    qt_sb = nc.alloc_sbuf_tensor("qt_sb", [128, HEADS, T], BF16).ap()   # [d, h, q]
    kt_sb = nc.alloc_sbuf_tensor("kt_sb", [128, NKT, 128], BF16).ap()   # [d, kt, k]
    v_aug = nc.alloc_sbuf_tensor("v_aug", [128, NKT, 132], BF16).ap()   # [k, kt, d+1]
    ident = nc.alloc_sbuf_tensor("ident", [128, 128], BF16).ap()

    with TileContext(nc) as tc:
        make_identity(nc, ident)
        nc.gpsimd.memset(v_aug[:, :, 128:129], 1.0)

        with (
            tc.tile_pool(name="ld", bufs=2) as ld,
            tc.tile_pool(name="ldb", bufs=3) as ldb,
            tc.tile_pool(name="st", bufs=2, space="PSUM") as stp,
            tc.tile_pool(name="ops", bufs=1, space="PSUM") as ops,
            tc.tile_pool(name="pt", bufs=4) as ptp,
            tc.tile_pool(name="osb", bufs=3) as osb,
            tc.tile_pool(name="rp", bufs=4) as rp,
        ):
            loaded = {}

            def load_k_dma(g):
                rows = slice(g * 512, (g + 1) * 512)
                kf = ld.tile([128, 4, 128], F32, tag="kf", bufs=4, name="kf")
                nc.sync.dma_start(
                    out=kf[:],
                    in_=k_in[rows, :].rearrange("(t p) d -> p t d", p=128),
                )
                loaded["k%d" % g] = kf

            def load_v_dma(g):
                rows = slice(g * 512, (g + 1) * 512)
                vf = ld.tile([128, 4, 128], F32, tag="vf", bufs=4, name="vf")
                nc.sync.dma_start(
                    out=vf[:],
                    in_=v_in[rows, :].rearrange("(t p) d -> p t d", p=128),
                )
                loaded["v%d" % g] = vf

            def v_copy(g):
                vf = loaded.pop("v%d" % g)
                nc.vector.tensor_copy(v_aug[:, 4 * g : 4 * g + 4, 0:128], vf[:])

            def load_q_dma(g, per_head=False, interleave_k=False):
                rows = slice(g * 512, (g + 1) * 512)
                if per_head:
                    qfs = []
                    for h in range(HEADS):
                        if interleave_k:
                            load_k_dma(h)
                        qfh = ld.tile([128, 4, 128], F32, tag="qf0", bufs=4, name="qfh")
                        nc.sync.dma_start(
                            out=qfh[:],
                            in_=q_in[rows, h * 128 : (h + 1) * 128].rearrange(
                                "(t p) d -> p t d", p=128
                            ),
                        )
                        qfs.append(qfh)
                    loaded[g] = lambda h: qfs[h][:, :, :]
                else:
                    if g in loaded:
                        return
                    qf = ld.tile([128, 4, 512], F32, tag="qf", bufs=3, name="qf")
                    nc.sync.dma_start(
                        out=qf[:],
                        in_=q_in[rows, :].rearrange("(t p) d -> p t d", p=128),
                    )
                    loaded[g] = lambda h: qf[:, :, h * 128 : (h + 1) * 128]

            def k_tr(g):
                kf = loaded.pop("k%d" % g)
                kb = ldb.tile([128, 4, 128], BF16, tag="kb", name="kb")
                nc.vector.tensor_copy(kb[:], kf[:])
                tr = stp.tile([128, 4, 128], BF16, tag="st", name="tr")
                for j2 in range(4):
                    nc.tensor.transpose(tr[:, j2, :], kb[:, j2, :], ident[:])
                nc.vector.tensor_copy(kt_sb[:, 4 * g : 4 * g + 4, :], tr[:])

            def q_tr(g, heads=range(HEADS)):
                """Cast to bf16 and PE-transpose Q group g into qt_sb."""
                q_slice = loaded[g]
                for h in heads:
                    qb = ldb.tile([128, 4, 128], BF16, tag="qb", name="qb")
                    nc.vector.tensor_copy(qb[:], q_slice(h))
                    tr = stp.tile([128, 4, 128], BF16, tag="st", name="tr")
                    for t in range(4):
                        nc.tensor.transpose(tr[:, t, :], qb[:, t, :], ident[:])
                    nc.vector.tensor_copy(
                        qt_sb[:, h, g * 512 : (g + 1) * 512], tr[:]
                    )

            # ---- software-pipelined attention ----
            pending = [None]

            def flush():
                if pending[0] is not None:
                    pending[0]()
                    pending[0] = None

            def attn_head(h, c, inject=None):
                q0 = c * W
                state = {}
                n_grp_total = (4 * c + 2) // 3 + 1  # full groups + band
                grp_idx = [0]

                def ensure_o():
                    if "o" in state:
                        return
                    state["o"] = [
                        ops.tile([128, 2, 130], F32, tag="o01", name="o01"),
                        ops.tile([128, 2, 130], F32, tag="o23", name="o23"),
                    ]
                    nc.vector.memset(state["o"][0][:], 0.0)
                    nc.vector.memset(state["o"][1][:], 0.0)

                def pv(j, t, lhsT):
                    nc.tensor.matmul(
                        state["o"][t // 2][:, t % 2, 0:129],
                        lhsT=lhsT,
                        rhs=v_aug[:, j, 0:129],
                        start=False,
                        stop=(j == 4 * c + t),
                        skip_group_check=True,
                    )

                def finish():
                    o_ts = state["o"]
                    ot = osb.tile([128, 4, 128], F32, tag="ot", name="ot")
                    for i in range(2):
                        r = rp.tile([128, 2, 1], F32, tag="r", name="r")
                        nc.vector.reciprocal(r[:], o_ts[i][:, :, 128:129])
                        nc.vector.tensor_mul(
                            ot[:, 2 * i : 2 * i + 2, :],
                            o_ts[i][:, :, 0:128],
                            r[:].broadcast_to([128, 2, 128]),
                        )
                    nc.sync.dma_start(
                        out=out[q0 : q0 + W, h * 128 : (h + 1) * 128].rearrange(
                            "(t p) d -> p t d", p=128
                        ),
                        in_=ot[:],
                    )

                # full k-tiles, groups of 3
                for gi, g0 in enumerate(range(0, 4 * c, 3)):
                    if inject and gi in inject:
                        inject[gi]()
                    grp = list(range(g0, min(g0 + 3, 4 * c)))
                    n = len(grp)
                    st = stp.tile([128, 3, 512], F32, tag="st", name="st")
                    pt = ptp.tile([128, 3, 512], BF16, tag="pt", name="pt")
                    for j2, j in enumerate(grp):
                        nc.tensor.matmul(
                            st[:, j2, :],
                            lhsT=kt_sb[:, j, :],
                            rhs=qt_sb[:, h, q0 : q0 + W],
                            start=True,
                            stop=True,
                        )
                    nc.scalar.activation(pt[:, 0:n, :], st[:, 0:n, :], EXP, scale=SCALE)
                    flush()

                    def emit_full(grp=grp, pt=pt):
                        ensure_o()
                        for j2, j in enumerate(grp):
                            for t in range(4):
                                pv(j, t, pt[:, j2, 128 * t : 128 * (t + 1)])

                    pending[0] = emit_full

                # band k-tiles (4c..4c+3), packed exact-width layout
                stb = stp.tile([128, BAND_W], F32, tag="st", name="stb")
                ptb = ptp.tile([128, BAND_W], BF16, tag="pt", name="ptb")
                for r in range(4):
                    j = 4 * c + r
                    off, wd = BAND_SLOTS[r]
                    nc.tensor.matmul(
                        stb[:, off : off + wd],
                        lhsT=kt_sb[:, j, :],
                        rhs=qt_sb[:, h, q0 + 128 * r : q0 + W],
                        start=True,
                        stop=True,
                    )
                nc.scalar.activation(ptb[:], stb[:], EXP, scale=SCALE)
                flush()

                def emit_band(ptb=ptb):
                    ensure_o()
                    for r in range(4):
                        # diagonal 128x128 block: zero where q < k
                        off, _ = BAND_SLOTS[r]
                        blk = ptb[:, off : off + 128]
                        nc.gpsimd.affine_select(
                            out=blk,
                            in_=blk,
                            compare_op=mybir.AluOpType.is_ge,
                            fill=0.0,
                            base=0,
                            pattern=[[1, 128]],
                            channel_multiplier=-1,
                        )
                    for r in range(4):  # off-diagonal blocks: not mask-gated
                        j = 4 * c + r
                        off, _ = BAND_SLOTS[r]
                        for t in range(r + 1, 4):
                            boff = off + 128 * (t - r)
                            pv(j, t, ptb[:, boff : boff + 128])
                    for r in range(4):  # diagonal blocks last
                        j = 4 * c + r
                        off, _ = BAND_SLOTS[r]
                        pv(j, r, ptb[:, off : off + 128])
                    finish()

                pending[0] = emit_band

            order = [3, 0, 2, 1]
            # interleaved dispatch: K groups and first-chunk per-head Q
            load_q_dma(order[0], per_head=True, interleave_k=True)
            for g in range(NC_CHUNK):
                load_v_dma(g)
            k_tr(0)
            for ci, c in enumerate(order):
                for h in range(HEADS):
                    inject = None
                    if ci == 0:
                        q_tr(c, heads=[h])
                        if h == 0:
                            v_copy(0), v_copy(1), v_copy(2)
                            inject = {1: lambda: k_tr(1), 2: lambda: k_tr(2), 3: lambda: k_tr(3)}
                        elif h == 1:
                            v_copy(3)
                    if h == 0 and ci + 1 < len(order):
                        load_q_dma(order[ci + 1])
                    if ci == 0 and h == 1:
                        load_q_dma(order[2])
                        load_q_dma(order[3])
                    if h == 1 and ci + 1 < len(order):
                        q_tr(order[ci + 1], heads=[0, 1])
                    if h == 2 and ci + 1 < len(order):
                        q_tr(order[ci + 1], heads=[2, 3])
                    attn_head(h, c, inject=inject)
            flush()

    nc.compile()
    return nc


_NC_CACHE = {}


def _get_nc():
    if "nc" not in _NC_CACHE:
        _NC_CACHE["nc"] = build_attention_nc()
    return _NC_CACHE["nc"]


def shard_inputs(query, key, value):
    in_maps = []
    for i in range(N_CORES):
        in_maps.append(
            {
                "query": np.ascontiguousarray(
                    query[:, i * HEADS * D : (i + 1) * HEADS * D], dtype=np.float32
                ),
                "key": np.ascontiguousarray(key[:, i * D : (i + 1) * D], dtype=np.float32),
                "value": np.ascontiguousarray(
                    value[:, i * D : (i + 1) * D], dtype=np.float32
                ),
            }
        )
    return in_maps


def kernel(query, key, value, _trace=False):
    nc = _get_nc()
    in_maps = shard_inputs(np.asarray(query), np.asarray(key), np.asarray(value))
    res = run_bass_kernel_spmd(nc, in_maps, core_ids=list(range(N_CORES)), trace=_trace)
    full = np.concatenate([res.results[i]["out"] for i in range(N_CORES)], axis=1)
    if _trace:
        kernel.last_result = res
    return full.astype(np.float32)


# revision 24
# speedup vs baseline: 1.0658x; 1.0018x over previous
"""Causal GQA attention on 8 TRN2 NeuronCores (head-sharded, no collectives).

Problem: NUM_TOKENS=2048, NUM_HEADS=32, HEAD_DIM=128, NUM_KV_HEADS=8, causal.
Sharding: core i holds KV head i and its 4 query heads (GQA group stays
together). Each core runs an independent flash-attention-style kernel:
  ST[k,q] = K @ Q^T  (bf16 matmuls, PSUM f32)
  PT      = exp(SCALE * ST)           (ACT, no max-subtraction: scores ~N(0,1))
  O[q, d+1] += PT_block^T @ [V | 1]   (ones column yields softmax denominators)
  out     = O[:, :d] / O[:, d]
The emission is software-pipelined: score matmuls of group i+1 are emitted
before the PV matmuls of group i, so the PE computes scores while ACT
exponentiates and never idles waiting for the activation.
"""

import numpy as np

import concourse.bass as bass
import concourse.bacc as bacc
import concourse.mybir as mybir
from concourse.tile import TileContext
from concourse.masks import make_identity
from concourse.bass_utils import run_bass_kernel_spmd

T = 2048          # tokens
D = 128           # head dim
HEADS = 4         # query heads per core
N_CORES = 8
W = 512           # q-chunk width
NKT = T // 128    # 16 k-tiles
NC_CHUNK = T // W
SCALE = D ** -0.5
F32 = mybir.dt.float32
BF16 = mybir.dt.bfloat16
EXP = mybir.ActivationFunctionType.Exp

# band packing: k-tile band offset r -> (packed col offset, width)
BAND_SLOTS = {0: (0, 512), 1: (512, 384), 3: (896, 128), 2: (1024, 256)}
BAND_W = 1280


def build_attention_nc():
    nc = bacc.Bacc("TRN2", target_bir_lowering=False, debug=False)

    q_in = nc.declare_dram_parameter("query", [T, HEADS * D], F32, isOutput=False)
    k_in = nc.declare_dram_parameter("key", [T, D], F32, isOutput=False)
    v_in = nc.declare_dram_parameter("value", [T, D], F32, isOutput=False)
    out = nc.declare_dram_parameter("out", [T, HEADS * D], F32, isOutput=True)

    # persistent SBUF tensors
    qt_sb = nc.alloc_sbuf_tensor("qt_sb", [128, HEADS, T], BF16).ap()   # [d, h, q]
    kt_sb = nc.alloc_sbuf_tensor("kt_sb", [128, NKT, 128], BF16).ap()   # [d, kt, k]
    v_aug = nc.alloc_sbuf_tensor("v_aug", [128, NKT, 132], BF16).ap()   # [k, kt, d+1]
    ident = nc.alloc_sbuf_tensor("ident", [128, 128], BF16).ap()

    with TileContext(nc) as tc:
        make_identity(nc, ident)
        nc.gpsimd.memset(v_aug[:, :, 128:129], 1.0)

        with (
            tc.tile_pool(name="ld", bufs=2) as ld,
            tc.tile_pool(name="ldb", bufs=3) as ldb,
            tc.tile_pool(name="st", bufs=2, space="PSUM") as stp,
            tc.tile_pool(name="ops", bufs=1, space="PSUM") as ops,
            tc.tile_pool(name="pt", bufs=4) as ptp,
            tc.tile_pool(name="osb", bufs=3) as osb,
            tc.tile_pool(name="rp", bufs=4) as rp,
        ):
            loaded = {}

            def load_k_dma(g):
                rows = slice(g * 512, (g + 1) * 512)
                kf = ld.tile([128, 4, 128], F32, tag="kf", bufs=4, name="kf")
                nc.sync.dma_start(
                    out=kf[:],
                    in_=k_in[rows, :].rearrange("(t p) d -> p t d", p=128),
                )
                loaded["k%d" % g] = kf

            def load_v_dma(g):
                rows = slice(g * 512, (g + 1) * 512)
                vf = ld.tile([128, 4, 128], F32, tag="vf", bufs=4, name="vf")
                nc.sync.dma_start(
                    out=vf[:],
                    in_=v_in[rows, :].rearrange("(t p) d -> p t d", p=128),
                )
                loaded["v%d" % g] = vf

            def v_copy(g):
                vf = loaded.pop("v%d" % g)
                nc.vector.tensor_copy(v_aug[:, 4 * g : 4 * g + 4, 0:128], vf[:])

            def load_q_dma(g, per_head=False, interleave_k=False):
                rows = slice(g * 512, (g + 1) * 512)
                if per_head:
                    qfs = []
                    for h in range(HEADS):
                        if interleave_k:
                            load_k_dma(h)
                        qfh = ld.tile([128, 4, 128], F32, tag="qf0", bufs=4, name="qfh")
                        nc.sync.dma_start(
                            out=qfh[:],
                            in_=q_in[rows, h * 128 : (h + 1) * 128].rearrange(
                                "(t p) d -> p t d", p=128
                            ),
                        )
                        qfs.append(qfh)
                    loaded[g] = lambda h: qfs[h][:, :, :]
                else:
                    if g in loaded:
                        return
                    qf = ld.tile([128, 4, 512], F32, tag="qf", bufs=3, name="qf")
                    nc.sync.dma_start(
                        out=qf[:],
                        in_=q_in[rows, :].rearrange("(t p) d -> p t d", p=128),
                    )
                    loaded[g] = lambda h: qf[:, :, h * 128 : (h + 1) * 128]

            def k_tr(g):
                kf = loaded.pop("k%d" % g)
                kb = ldb.tile([128, 4, 128], BF16, tag="kb", name="kb")
                nc.vector.tensor_copy(kb[:], kf[:])
                tr = stp.tile([128, 4, 128], BF16, tag="st", name="tr")
                for j2 in range(4):
                    nc.tensor.transpose(tr[:, j2, :], kb[:, j2, :], ident[:])
                nc.vector.tensor_copy(kt_sb[:, 4 * g : 4 * g + 4, :], tr[:])

            def q_tr(g, heads=range(HEADS)):
                """Cast to bf16 and PE-transpose Q group g into qt_sb."""
                q_slice = loaded[g]
                for h in heads:
                    qb = ldb.tile([128, 4, 128], BF16, tag="qb", name="qb")
                    nc.vector.tensor_copy(qb[:], q_slice(h))
                    tr = stp.tile([128, 4, 128], BF16, tag="st", name="tr")
                    for t in range(4):
                        nc.tensor.transpose(tr[:, t, :], qb[:, t, :], ident[:])
                    nc.vector.tensor_copy(
                        qt_sb[:, h, g * 512 : (g + 1) * 512], tr[:]
                    )

            # ---- software-pipelined attention ----
            pending = [None]

            def flush():
                if pending[0] is not None:
                    pending[0]()
                    pending[0] = None

            def attn_head(h, c, inject=None):
                q0 = c * W
                state = {}
                n_grp_total = (4 * c + 2) // 3 + 1  # full groups + band
                grp_idx = [0]

                def ensure_o():
                    if "o" in state:
                        return
                    state["o"] = [
                        ops.tile([128, 2, 130], F32, tag="o01", name="o01"),
                        ops.tile([128, 2, 130], F32, tag="o23", name="o23"),
                    ]
                    nc.vector.memset(state["o"][0][:], 0.0)
                    nc.vector.memset(state["o"][1][:], 0.0)

                def pv(j, t, lhsT):
                    nc.tensor.matmul(
                        state["o"][t // 2][:, t % 2, 0:129],
                        lhsT=lhsT,
                        rhs=v_aug[:, j, 0:129],
                        start=False,
                        stop=(j == 4 * c + t),
                        skip_group_check=True,
                    )

                def finish():
                    o_ts = state["o"]
                    ot = osb.tile([128, 4, 128], F32, tag="ot", name="ot")
                    for i in range(2):
                        r = rp.tile([128, 2, 1], F32, tag="r", name="r")
                        nc.vector.reciprocal(r[:], o_ts[i][:, :, 128:129])
                        nc.vector.tensor_mul(
                            ot[:, 2 * i : 2 * i + 2, :],
                            o_ts[i][:, :, 0:128],
                            r[:].broadcast_to([128, 2, 128]),
                        )
                    nc.sync.dma_start(
                        out=out[q0 : q0 + W, h * 128 : (h + 1) * 128].rearrange(
                            "(t p) d -> p t d", p=128
                        ),
                        in_=ot[:],
                    )

                # full k-tiles, groups of 3
                for gi, g0 in enumerate(range(0, 4 * c, 3)):
                    if inject and gi in inject:
                        inject[gi]()
                    grp = list(range(g0, min(g0 + 3, 4 * c)))
                    n = len(grp)
                    st = stp.tile([128, 3, 512], F32, tag="st", name="st")
                    pt = ptp.tile([128, 3, 512], BF16, tag="pt", name="pt")
                    for j2, j in enumerate(grp):
                        nc.tensor.matmul(
                            st[:, j2, :],
                            lhsT=kt_sb[:, j, :],
                            rhs=qt_sb[:, h, q0 : q0 + W],
                            start=True,
                            stop=True,
                        )
                    nc.scalar.activation(pt[:, 0:n, :], st[:, 0:n, :], EXP, scale=SCALE)
                    flush()

                    def emit_full(grp=grp, pt=pt):
                        ensure_o()
                        for j2, j in enumerate(grp):
                            for t in range(4):
                                pv(j, t, pt[:, j2, 128 * t : 128 * (t + 1)])

                    pending[0] = emit_full

                # band k-tiles (4c..4c+3), packed exact-width layout
                stb = stp.tile([128, BAND_W], F32, tag="st", name="stb")
                ptb = ptp.tile([128, BAND_W], BF16, tag="pt", name="ptb")
                for r in range(4):
                    j = 4 * c + r
                    off, wd = BAND_SLOTS[r]
                    nc.tensor.matmul(
                        stb[:, off : off + wd],
                        lhsT=kt_sb[:, j, :],
                        rhs=qt_sb[:, h, q0 + 128 * r : q0 + W],
                        start=True,
                        stop=True,
                    )
                nc.scalar.activation(ptb[:], stb[:], EXP, scale=SCALE)
                flush()

                def emit_band(ptb=ptb):
                    ensure_o()
                    for r in range(4):
                        # diagonal 128x128 block: zero where q < k
                        off, _ = BAND_SLOTS[r]
                        blk = ptb[:, off : off + 128]
                        nc.gpsimd.affine_select(
                            out=blk,
                            in_=blk,
                            compare_op=mybir.AluOpType.is_ge,
                            fill=0.0,
                            base=0,
                            pattern=[[1, 128]],
                            channel_multiplier=-1,
                        )
                    for r in range(4):  # off-diagonal blocks: not mask-gated
                        j = 4 * c + r
                        off, _ = BAND_SLOTS[r]
                        for t in range(r + 1, 4):
                            boff = off + 128 * (t - r)
                            pv(j, t, ptb[:, boff : boff + 128])
                    for r in range(4):  # diagonal blocks last
                        j = 4 * c + r
                        off, _ = BAND_SLOTS[r]
                        pv(j, r, ptb[:, off : off + 128])
                    finish()

                pending[0] = emit_band

            order = [3, 2, 0, 1]
            # interleaved dispatch: K groups and first-chunk per-head Q
            load_q_dma(order[0], per_head=True, interleave_k=True)
            for g in range(NC_CHUNK):
                load_v_dma(g)
            k_tr(0)
            for ci, c in enumerate(order):
                for h in range(HEADS):
                    inject = None
                    if ci == 0:
                        q_tr(c, heads=[h])
                        if h == 0:
                            v_copy(0), v_copy(1), v_copy(2)
                            inject = {1: lambda: k_tr(1), 2: lambda: k_tr(2), 3: lambda: k_tr(3)}
                        elif h == 1:
                            v_copy(3)
                    if h == 0 and ci + 1 < len(order):
                        load_q_dma(order[ci + 1])
                    if ci == 0 and h == 1:
                        load_q_dma(order[2])
                        load_q_dma(order[3])
                    if h == 1 and ci + 1 < len(order):
                        q_tr(order[ci + 1], heads=[0, 1])
                    if h == 2 and ci + 1 < len(order):
                        q_tr(order[ci + 1], heads=[2, 3])
                    attn_head(h, c, inject=inject)
            flush()

    nc.compile()
    return nc


_NC_CACHE = {}


def _get_nc():
    if "nc" not in _NC_CACHE:
        _NC_CACHE["nc"] = build_attention_nc()
    return _NC_CACHE["nc"]


def shard_inputs(query, key, value):
    in_maps = []
    for i in range(N_CORES):
        in_maps.append(
            {
                "query": np.ascontiguousarray(
                    query[:, i * HEADS * D : (i + 1) * HEADS * D], dtype=np.float32
                ),
                "key": np.ascontiguousarray(key[:, i * D : (i + 1) * D], dtype=np.float32),
                "value": np.ascontiguousarray(
                    value[:, i * D : (i + 1) * D], dtype=np.float32
                ),
            }
        )
    return in_maps


def kernel(query, key, value, _trace=False):
    nc = _get_nc()
    in_maps = shard_inputs(np.asarray(query), np.asarray(key), np.asarray(value))
    res = run_bass_kernel_spmd(nc, in_maps, core_ids=list(range(N_CORES)), trace=_trace)
    full = np.concatenate([res.results[i]["out"] for i in range(N_CORES)], axis=1)
    if _trace:
        kernel.last_result = res
    return full.astype(np.float32)


# revision 25
# speedup vs baseline: 1.1350x; 1.0649x over previous
"""Causal GQA attention on 8 TRN2 NeuronCores (head-sharded, no collectives).

Problem: NUM_TOKENS=2048, NUM_HEADS=32, HEAD_DIM=128, NUM_KV_HEADS=8, causal.
Sharding: core i holds KV head i and its 4 query heads (GQA group stays
together). Each core runs an independent flash-attention-style kernel:
  ST[k,q] = K @ Q^T  (bf16 matmuls, PSUM f32)
  PT      = exp(SCALE * ST)           (ACT, no max-subtraction: scores ~N(0,1))
  O[q, d+1] += PT_block^T @ [V | 1]   (ones column yields softmax denominators)
  out     = O[:, :d] / O[:, d]
The emission is software-pipelined: score matmuls of group i+1 are emitted
before the PV matmuls of group i, so the PE computes scores while ACT
exponentiates and never idles waiting for the activation.
"""

import numpy as np

import concourse.bass as bass
import concourse.bacc as bacc
import concourse.mybir as mybir
from concourse.tile import TileContext
from concourse.masks import make_identity
from concourse.bass_utils import run_bass_kernel_spmd

T = 2048          # tokens
D = 128           # head dim
HEADS = 4         # query heads per core
N_CORES = 8
W = 512           # q-chunk width
NKT = T // 128    # 16 k-tiles
NC_CHUNK = T // W
SCALE = D ** -0.5
F32 = mybir.dt.float32
BF16 = mybir.dt.bfloat16
EXP = mybir.ActivationFunctionType.Exp

# band packing: k-tile band offset r -> (packed col offset, width)
BAND_SLOTS = {0: (0, 512), 1: (512, 384), 3: (896, 128), 2: (1024, 256)}
BAND_W = 1280


def build_attention_nc():
    nc = bacc.Bacc("TRN2", target_bir_lowering=False, debug=False)

    q_in = nc.declare_dram_parameter("query", [T, HEADS * D], F32, isOutput=False)
    k_in = nc.declare_dram_parameter("key", [T, D], F32, isOutput=False)
    v_in = nc.declare_dram_parameter("value", [T, D], F32, isOutput=False)
    out = nc.declare_dram_parameter("out", [T, HEADS * D], F32, isOutput=True)

    # persistent SBUF tensors
    qt_sb = nc.alloc_sbuf_tensor("qt_sb", [128, HEADS, T], BF16).ap()   # [d, h, q]
    kt_sb = nc.alloc_sbuf_tensor("kt_sb", [128, NKT, 128], BF16).ap()   # [d, kt, k]
    v_aug = nc.alloc_sbuf_tensor("v_aug", [128, NKT, 132], BF16).ap()   # [k, kt, d+1]
    ident = nc.alloc_sbuf_tensor("ident", [128, 128], BF16).ap()

    with TileContext(nc) as tc:
        make_identity(nc, ident)
        nc.gpsimd.memset(v_aug[:, :, 128:129], 1.0)

        with (
            tc.tile_pool(name="ld", bufs=2) as ld,
            tc.tile_pool(name="ldb", bufs=3) as ldb,
            tc.tile_pool(name="st", bufs=2, space="PSUM") as stp,
            tc.tile_pool(name="ops", bufs=1, space="PSUM") as ops,
            tc.tile_pool(name="pt", bufs=4) as ptp,
            tc.tile_pool(name="osb", bufs=3) as osb,
            tc.tile_pool(name="rp", bufs=4) as rp,
        ):
            loaded = {}

            def load_k_dma(g):
                rows = slice(g * 512, (g + 1) * 512)
                kf = ld.tile([128, 4, 128], F32, tag="kf", bufs=4, name="kf")
                nc.sync.dma_start(
                    out=kf[:],
                    in_=k_in[rows, :].rearrange("(t p) d -> p t d", p=128),
                )
                loaded["k%d" % g] = kf

            def load_v_dma(g):
                rows = slice(g * 512, (g + 1) * 512)
                vf = ld.tile([128, 4, 128], F32, tag="vf", bufs=4, name="vf")
                nc.sync.dma_start(
                    out=vf[:],
                    in_=v_in[rows, :].rearrange("(t p) d -> p t d", p=128),
                )
                loaded["v%d" % g] = vf

            def v_copy(g):
                vf = loaded.pop("v%d" % g)
                nc.vector.tensor_copy(v_aug[:, 4 * g : 4 * g + 4, 0:128], vf[:])

            def load_q_dma(g, per_head=False, interleave_k=False):
                rows = slice(g * 512, (g + 1) * 512)
                if per_head:
                    qfs = []

                    def q_head_dma(h):
                        qfh = ld.tile([128, 4, 128], F32, tag="qf0", bufs=4, name="qfh")
                        nc.sync.dma_start(
                            out=qfh[:],
                            in_=q_in[rows, h * 128 : (h + 1) * 128].rearrange(
                                "(t p) d -> p t d", p=128
                            ),
                        )
                        qfs.append(qfh)

                    if interleave_k:
                        load_k_dma(0)
                    q_head_dma(0)
                    if interleave_k:
                        for g2 in range(1, NC_CHUNK):
                            load_k_dma(g2)
                        for g2 in range(NC_CHUNK):
                            load_v_dma(g2)
                    for h in range(1, HEADS):
                        q_head_dma(h)
                    loaded[g] = lambda h: qfs[h][:, :, :]
                else:
                    if g in loaded:
                        return
                    qf = ld.tile([128, 4, 512], F32, tag="qf", bufs=3, name="qf")
                    nc.sync.dma_start(
                        out=qf[:],
                        in_=q_in[rows, :].rearrange("(t p) d -> p t d", p=128),
                    )
                    loaded[g] = lambda h: qf[:, :, h * 128 : (h + 1) * 128]

            def k_tr(g):
                kf = loaded.pop("k%d" % g)
                kb = ldb.tile([128, 4, 128], BF16, tag="kb", name="kb")
                nc.vector.tensor_copy(kb[:], kf[:])
                tr = stp.tile([128, 4, 128], BF16, tag="st", name="tr")
                for j2 in range(4):
                    nc.tensor.transpose(tr[:, j2, :], kb[:, j2, :], ident[:])
                nc.vector.tensor_copy(kt_sb[:, 4 * g : 4 * g + 4, :], tr[:])

            def q_tr(g, heads=range(HEADS)):
                """Cast to bf16 and PE-transpose Q group g into qt_sb."""
                q_slice = loaded[g]
                for h in heads:
                    qb = ldb.tile([128, 4, 128], BF16, tag="qb", name="qb")
                    nc.vector.tensor_copy(qb[:], q_slice(h))
                    tr = stp.tile([128, 4, 128], BF16, tag="st", name="tr")
                    for t in range(4):
                        nc.tensor.transpose(tr[:, t, :], qb[:, t, :], ident[:])
                    nc.vector.tensor_copy(
                        qt_sb[:, h, g * 512 : (g + 1) * 512], tr[:]
                    )

            # ---- software-pipelined attention (depth 2) ----
            pending = []

            def push(fn):
                pending.append(fn)
                while len(pending) > 2:
                    pending.pop(0)()

            def flush_all():
                while pending:
                    pending.pop(0)()

            def attn_head(h, c, inject=None):
                q0 = c * W
                state = {}
                n_grp_total = (4 * c + 2) // 3 + 1  # full groups + band
                grp_idx = [0]

                def ensure_o():
                    if "o" in state:
                        return
                    state["o"] = [
                        ops.tile([128, 2, 130], F32, tag="o01", name="o01"),
                        ops.tile([128, 2, 130], F32, tag="o23", name="o23"),
                    ]
                    nc.vector.memset(state["o"][0][:], 0.0)
                    nc.vector.memset(state["o"][1][:], 0.0)

                def pv(j, t, lhsT):
                    nc.tensor.matmul(
                        state["o"][t // 2][:, t % 2, 0:129],
                        lhsT=lhsT,
                        rhs=v_aug[:, j, 0:129],
                        start=False,
                        stop=(j == 4 * c + t),
                        skip_group_check=True,
                    )

                def finish():
                    o_ts = state["o"]
                    ot = osb.tile([128, 4, 128], F32, tag="ot", name="ot")
                    for i in range(2):
                        r = rp.tile([128, 2, 1], F32, tag="r", name="r")
                        nc.vector.reciprocal(r[:], o_ts[i][:, :, 128:129])
                        nc.vector.tensor_mul(
                            ot[:, 2 * i : 2 * i + 2, :],
                            o_ts[i][:, :, 0:128],
                            r[:].broadcast_to([128, 2, 128]),
                        )
                    nc.sync.dma_start(
                        out=out[q0 : q0 + W, h * 128 : (h + 1) * 128].rearrange(
                            "(t p) d -> p t d", p=128
                        ),
                        in_=ot[:],
                    )

                # full k-tiles, groups of 3
                for gi, g0 in enumerate(range(0, 4 * c, 3)):
                    if inject and gi in inject:
                        inject[gi]()
                    grp = list(range(g0, min(g0 + 3, 4 * c)))
                    n = len(grp)
                    st = stp.tile([128, 3, 512], F32, tag="st", name="st")
                    pt = ptp.tile([128, 3, 512], BF16, tag="pt", name="pt")
                    for j2, j in enumerate(grp):
                        nc.tensor.matmul(
                            st[:, j2, :],
                            lhsT=kt_sb[:, j, :],
                            rhs=qt_sb[:, h, q0 : q0 + W],
                            start=True,
                            stop=True,
                        )
                    nc.scalar.activation(pt[:, 0:n, :], st[:, 0:n, :], EXP, scale=SCALE)

                    def emit_full(grp=grp, pt=pt):
                        ensure_o()
                        for j2, j in enumerate(grp):
                            for t in range(4):
                                pv(j, t, pt[:, j2, 128 * t : 128 * (t + 1)])

                    push(emit_full)

                # band k-tiles (4c..4c+3), packed exact-width layout
                stb = stp.tile([128, BAND_W], F32, tag="st", name="stb")
                ptb = ptp.tile([128, BAND_W], BF16, tag="pt", name="ptb")
                for r in range(4):
                    j = 4 * c + r
                    off, wd = BAND_SLOTS[r]
                    nc.tensor.matmul(
                        stb[:, off : off + wd],
                        lhsT=kt_sb[:, j, :],
                        rhs=qt_sb[:, h, q0 + 128 * r : q0 + W],
                        start=True,
                        stop=True,
                    )
                nc.scalar.activation(ptb[:], stb[:], EXP, scale=SCALE)

                def emit_band(ptb=ptb):
                    ensure_o()
                    for r in range(4):
                        # diagonal 128x128 block: zero where q < k
                        off, _ = BAND_SLOTS[r]
                        blk = ptb[:, off : off + 128]
                        nc.gpsimd.affine_select(
                            out=blk,
                            in_=blk,
                            compare_op=mybir.AluOpType.is_ge,
                            fill=0.0,
                            base=0,
                            pattern=[[1, 128]],
                            channel_multiplier=-1,
                        )
                    for r in range(4):  # off-diagonal blocks: not mask-gated
                        j = 4 * c + r
                        off, _ = BAND_SLOTS[r]
                        for t in range(r + 1, 4):
                            boff = off + 128 * (t - r)
                            pv(j, t, ptb[:, boff : boff + 128])
                    for r in range(4):  # diagonal blocks last
                        j = 4 * c + r
                        off, _ = BAND_SLOTS[r]
                        pv(j, r, ptb[:, off : off + 128])
                    finish()

                push(emit_band)

            order = [3, 2, 0, 1]
            # dispatch by first-need: K0, Q3h0, K1-3, V0-3, Q3h1-3
            load_q_dma(order[0], per_head=True, interleave_k=True)
            k_tr(0)
            for ci, c in enumerate(order):
                for h in range(HEADS):
                    inject = None
                    if ci == 0:
                        q_tr(c, heads=[h])
                        if h == 0:
                            v_copy(0), v_copy(1), v_copy(2)
                            inject = {1: lambda: k_tr(1), 2: lambda: k_tr(2), 3: lambda: k_tr(3)}
                        elif h == 1:
                            v_copy(3)
                    if h == 0 and ci + 1 < len(order):
                        load_q_dma(order[ci + 1])
                    if ci == 0 and h == 1:
                        load_q_dma(order[2])
                        load_q_dma(order[3])
                    if h == 1 and ci + 1 < len(order):
                        q_tr(order[ci + 1], heads=[0, 1])
                    if h == 2 and ci + 1 < len(order):
                        q_tr(order[ci + 1], heads=[2, 3])
                    attn_head(h, c, inject=inject)
            flush_all()

    nc.compile()
    return nc


_NC_CACHE = {}


def _get_nc():
    if "nc" not in _NC_CACHE:
        _NC_CACHE["nc"] = build_attention_nc()
    return _NC_CACHE["nc"]


def shard_inputs(query, key, value):
    in_maps = []
    for i in range(N_CORES):
        in_maps.append(
            {
                "query": np.ascontiguousarray(
                    query[:, i * HEADS * D : (i + 1) * HEADS * D], dtype=np.float32
                ),
                "key": np.ascontiguousarray(key[:, i * D : (i + 1) * D], dtype=np.float32),
                "value": np.ascontiguousarray(
                    value[:, i * D : (i + 1) * D], dtype=np.float32
                ),
            }
        )
    return in_maps


def kernel(query, key, value, _trace=False):
    nc = _get_nc()
    in_maps = shard_inputs(np.asarray(query), np.asarray(key), np.asarray(value))
    res = run_bass_kernel_spmd(nc, in_maps, core_ids=list(range(N_CORES)), trace=_trace)
    full = np.concatenate([res.results[i]["out"] for i in range(N_CORES)], axis=1)
    if _trace:
        kernel.last_result = res
    return full.astype(np.float32)
